# revision 31
# baseline (speedup 1.0000x reference)
"""Trainium2 Bass kernel for batched channel-attention (nn_Attention_28071906246667).

Reference computation (per batch element n, with xT = batch_flat[n] of shape [C, HW]):
    x   = xT.T                                  # [HW, C]
    Q   = x @ Wq.T + bq ; K, V likewise         # [HW, D]
    S   = Q.T @ K                               # [D, D]
    att = softmax(S, axis=-1)
    out = att @ V.T                             # [D, HW]

Key algebraic restructuring (halves FLOPs, avoids materializing Q/K/V):
    G = x.T x  (Gram over channels), m = column sums of x. Then
      S   = Wq G Wk.T + (Wq m) bk.T + bq (Wk m).T + HW bq bk.T
          = Wq_aug @ U,   U = [G m; m.T HW] @ WkT_aug
      out = att @ V.T = (att_unnorm @ Wv) @ xT + att_unnorm @ bv, normalized at the end.

Sharding: pure data parallel, batch N=16 -> 2 per core across 8 cores.
All matmuls run in float32r (fp32 with 11 explicit mantissa bits, full PE speed).
f32r ISA restrictions honored: moving operand & psum dst innermost counts even,
dst starts at partition 0.
"""

import numpy as np

N, C, HW, D = 16, 512, 3136, 512
NCORES = 8
NPC = N // NCORES          # batch elements per core
CT = C // 128              # 4 c partition tiles
DT = D // 128              # 4 d partition tiles
KT = 25                    # s k-tiles: 24 x 128 + 1 x 64
KT_ROWS = [128] * 24 + [64]
OC = 448                   # out-phase s chunk
NOC = HW // OC             # 7 chunks; DMA'd per 3+4-chunk pieces


def _f32r_round(a: np.ndarray) -> np.ndarray:
    """Round fp32 to float32r (11 explicit mantissa bits, round-to-nearest)."""
    bits = np.ascontiguousarray(a, dtype=np.float32).view(np.uint32)
    bits = (bits + np.uint32(0x800)) & np.uint32(0xFFFFF000)
    return bits.view(np.float32)


def _build_nc():
    import concourse.mybir as mybir
    from concourse import bacc
    from concourse.tile import TileContext

    f32 = mybir.dt.float32
    f32r = mybir.dt.float32r
    MUL = mybir.AluOpType.mult
    ADD = mybir.AluOpType.add

    nc = bacc.Bacc("TRN2", target_bir_lowering=False, debug=False,
                   num_devices=NCORES)

    x_ext = nc.declare_dram_parameter("x", [NPC, C, HW], f32r, isOutput=False)
    xs_ext = nc.declare_dram_parameter("xs", [NPC, HW, C], f32r, isOutput=False)
    m_ext = nc.declare_dram_parameter("m", [NPC, C], f32r, isOutput=False)
    wqT_ext = nc.declare_dram_parameter("wqT", [C + 1, D], f32r, isOutput=False)
    wkT_ext = nc.declare_dram_parameter("wkT", [C + 1, D], f32r, isOutput=False)
    wv_ext = nc.declare_dram_parameter("wv", [D, C], f32r, isOutput=False)
    id_ext = nc.declare_dram_parameter("ident", [128, 128], f32r, isOutput=False)
    bkb_ext = nc.declare_dram_parameter("bkb", [128, D], f32r, isOutput=False)
    bvb_ext = nc.declare_dram_parameter("bvb", [128, D], f32r, isOutput=False)
    out_ext = nc.declare_dram_parameter("out", [NPC, D, HW], f32, isOutput=True)

    with TileContext(nc) as tc:
        with (
            tc.tile_pool(name="wpool", bufs=1) as wp,
            tc.tile_pool(name="xbig", bufs=2) as xb,
            tc.tile_pool(name="work", bufs=1) as wk,
            tc.tile_pool(name="small", bufs=2) as sm,
            tc.tile_pool(name="outsb", bufs=4) as osb,
            tc.tile_pool(name="acc4", bufs=4, space="PSUM") as acc4,
            tc.tile_pool(name="stage", bufs=3, space="PSUM") as stg,
            tc.tile_pool(name="extra", bufs=1, space="PSUM") as xtr,
        ):
            # ---- weights (loaded once) ----
            wq_t = wp.tile([128, CT, D], f32r, tag="wq")
            wq4 = wp.tile([1, D], f32r, tag="wq4")
            wk_t = wp.tile([128, CT, D], f32r, tag="wk")
            wk4 = wp.tile([1, D], f32r, tag="wk4")
            wv_t = wp.tile([128, DT, C], f32r, tag="wv")
            ident = wp.tile([128, 128], f32r, tag="ident")
            bkb = wp.tile([128, D], f32r, tag="bkb")
            bvb = wp.tile([128, D], f32r, tag="bvb")
            hw_t = wp.tile([1, 2], f32r, tag="hw")
            nc.vector.memset(hw_t[:].bitcast(f32), float(HW))

            def load_weights():
                nc.sync.dma_start(out=wk4[:], in_=wkT_ext[C:C + 1, :])
                nc.sync.dma_start(out=wk_t[:],
                                  in_=wkT_ext[0:C, :].rearrange("(ci p) d -> p ci d", p=128))
                nc.sync.dma_start(out=wq_t[:],
                                  in_=wqT_ext[0:C, :].rearrange("(ci p) d -> p ci d", p=128))
                nc.sync.dma_start(out=wq4[:], in_=wqT_ext[C:C + 1, :])
                nc.sync.dma_start(out=bkb[:], in_=bkb_ext[:])
                nc.sync.dma_start(out=ident[:], in_=id_ext[:])
                nc.sync.dma_start(out=wv_t[:],
                                  in_=wv_ext[:].rearrange("(ci p) d -> p ci d", p=128))
                nc.sync.dma_start(out=bvb[:], in_=bvb_ext[:])

            st = {}

            # ---- PE warm-up: keep the HAM activity monitor busy during the
            # DMA lead-in so real matmuls start at 2.4 GHz ----
            warm_sb = wp.tile([128, 128], mybir.dt.bfloat16, tag="warm")
            warm_ps = xtr.tile([128, 128], f32, tag="extra", name="warm_ps")
            nc.vector.memset(warm_sb[:], 0.0)
            for wi in range(26):
                nc.tensor.matmul(warm_ps[:], warm_sb[:], warm_sb[:],
                                 start=True, stop=True)

            def phase_load_xs(n):
                xs = xb.tile([128, KT, C], f32r, tag="xbig", name=f"xs{n}")
                kt0 = 0
                for ci_, nkt in enumerate([1, 1, 1, 2, 2, 4, 4, 5, 4, 1] if n == 0 else [4, 4, 4, 4, 4, 4, 1]):
                    s0, s1 = kt0 * 128, min((kt0 + nkt) * 128, HW)
                    eng = nc.sync
                    if s1 - s0 == nkt * 128:
                        eng.dma_start(
                            out=xs[:, kt0:kt0 + nkt, :],
                            in_=xs_ext[n, s0:s1, :].rearrange("(k p) c -> p k c", p=128))
                    else:
                        eng.dma_start(out=xs[:s1 - s0, kt0, :],
                                      in_=xs_ext[n, s0:s1, :])
                    kt0 += nkt
                m_r = sm.tile([128, CT], f32r, tag="mr", name=f"mr{n}")
                nc.sync.dma_start(out=m_r[:],
                                  in_=m_ext[n, :].rearrange("(ci c) -> c ci", c=128))
                st[n] = {"xs": xs, "m_r": m_r}

            def phase_load_xt(n, halves=False):
                xt = xb.tile([128, CT, HW], f32r, tag="xbig", name=f"xt{n}")
                if halves:
                    for hoff, hw_ in [(0, 1568), (1568, 1568)]:
                        for ci in range(CT):
                            nc.sync.dma_start(out=xt[:, ci, hoff:hoff + hw_],
                                              in_=x_ext[n, ci * 128:(ci + 1) * 128,
                                                        hoff:hoff + hw_])
                else:
                    for ci in range(CT):
                        nc.sync.dma_start(out=xt[:, ci, :],
                                          in_=x_ext[n, ci * 128:(ci + 1) * 128, :])
                st[n]["xt"] = xt

            G_COLS = {0: (0, 512), 1: (128, 512), 2: (256, 512), 3: (0, 512)}

            def phase_G(n):
                xs = st[n]["xs"]
                g_ps = [acc4.tile([128, 512], f32, tag="acc4", name=f"g_ps{n}_{j}")
                        for j in range(CT)]
                st[n]["g_ps"] = g_ps
                for kt in range(KT):
                    rows = KT_ROWS[kt]
                    for j in range(CT):
                        c0, c1 = G_COLS[j]
                        nc.tensor.matmul(
                            g_ps[j][:, c0:c1],
                            xs[:rows, kt, j * 128:(j + 1) * 128],
                            xs[:rows, kt, c0:c1],
                            start=(kt == 0), stop=(kt == KT - 1))
                        yield

            def phase_mid(n):
                g_ps = st[n]["g_ps"]
                m_r = st[n]["m_r"]
                u4_ps = xtr.tile([1, 512], f32, tag="extra", name=f"u4_ps{n}")
                for k in range(CT):
                    nc.tensor.matmul(u4_ps[:], m_r[:, k:k + 1], wk_t[:, k, :],
                                     start=(k == 0), stop=False)
                nc.tensor.matmul(u4_ps[:], hw_t[0:1, 0:1], wk4[:],
                                 start=False, stop=True)
                g = wk.tile([128, CT, 512], f32r, tag="g", name=f"g{n}")
                for j in range(CT):
                    c0, c1 = G_COLS[j]
                    nc.vector.tensor_copy(g[:, j, c0:c1], g_ps[j][:, c0:c1])
                gfill = stg.tile([128, 512], f32r, tag="stage", name=f"gfill{n}")
                for fi, (bi, bj) in enumerate([(1, 0), (2, 0), (2, 1)]):
                    nc.tensor.transpose(gfill[:, fi * 128:(fi + 1) * 128],
                                        g[:, bj, bi * 128:(bi + 1) * 128], ident[:])
                for fi, (bi, bj) in enumerate([(1, 0), (2, 0), (2, 1)]):
                    nc.vector.tensor_copy(g[:, bi, bj * 128:(bj + 1) * 128],
                                          gfill[:, fi * 128:(fi + 1) * 128])

                # U = G~ @ WkT_aug
                u = wk.tile([128, CT, D], f32r, tag="u", name=f"u{n}")
                u4 = wk.tile([1, D], f32r, tag="u4", name=f"u4{n}")
                u_ps = [acc4.tile([128, 512], f32, tag="acc4", name=f"u_ps{n}_{j}")
                        for j in range(CT)]
                for ki, k in enumerate([0, 3, 1, 2]):
                    for j in range(CT):
                        nc.tensor.matmul(u_ps[j][:], g[:, k, j * 128:(j + 1) * 128],
                                         wk_t[:, k, :], start=(ki == 0), stop=(ki == CT - 1))
                for j in range(CT):
                    nc.vector.scalar_tensor_tensor(
                        u[:, j, :], bkb[:], m_r[:, j:j + 1], u_ps[j][:],
                        op0=MUL, op1=ADD)
                nc.vector.tensor_copy(u4[:], u4_ps[:])

                # S = Wq_aug @ U_aug ; softmax
                s_ps = [acc4.tile([128, 512], f32, tag="acc4", name=f"s_ps{n}_{j}")
                        for j in range(DT)]
                for jd in range(DT):
                    for k in range(CT + 1):
                        lt = wq_t[:, k, :] if k < CT else wq4[:]
                        rhs = u[:, k, :] if k < CT else u4[:]
                        nc.tensor.matmul(s_ps[jd][:], lt[:, jd * 128:(jd + 1) * 128],
                                         rhs, start=(k == 0), stop=(k == CT))

                negmax = sm.tile([128, DT], f32, tag="negmax", name=f"negmax{n}")
                sumexp = sm.tile([128, DT], f32, tag="sumexp", name=f"sumexp{n}")
                recip = sm.tile([128, DT], f32, tag="recip", name=f"recip{n}")
                expS = wk.tile([128, DT, D], f32r, tag="expS", name=f"expS{n}")
                for jd in range(DT):
                    nc.vector.reduce_max(negmax[:, jd:jd + 1], s_ps[jd][:],
                                         axis=mybir.AxisListType.X, negate=True)
                    nc.scalar.activation(expS[:, jd, :], s_ps[jd][:],
                                         mybir.ActivationFunctionType.Exp,
                                         bias=negmax[:, jd:jd + 1], scale=1.0,
                                         accum_out=sumexp[:, jd:jd + 1])
                nc.vector.reciprocal(recip[:], sumexp[:])
                bias_d = sm.tile([128, DT], f32, tag="bias_d", name=f"bias_d{n}")
                bias_dummy = sm.tile([128, 512], f32, tag="bias_dummy",
                                     name=f"bias_dummy{n}")
                for jd in range(DT):
                    nc.vector.scalar_tensor_tensor(
                        bias_dummy[:], expS[:, jd, :], 1.0, bvb[:],
                        op0=MUL, op1=MUL, accum_out=bias_d[:, jd:jd + 1])

                # attT via transposes
                attT = wk.tile([128, DT, D], f32r, tag="attT", name=f"attT{n}")
                for je in range(DT):
                    at = stg.tile([128, 512], f32r, tag="stage", name=f"at{n}_{je}")
                    for jd in range(DT):
                        nc.tensor.transpose(at[:, jd * 128:(jd + 1) * 128],
                                            expS[:, jd, je * 128:(je + 1) * 128],
                                            ident[:])
                    nc.vector.tensor_copy(attT[:, je, :], at[:])

                # B^T = Wv^T @ attT
                bT = wk.tile([128, CT, D], f32r, tag="bT", name=f"bT{n}")
                b_ps = [acc4.tile([128, 512], f32, tag="acc4", name=f"b_ps{n}_{j}")
                        for j in range(CT)]
                for je in range(DT):
                    for jc in range(CT):
                        nc.tensor.matmul(b_ps[jc][:], wv_t[:, je, jc * 128:(jc + 1) * 128],
                                         attT[:, je, :], start=(je == 0), stop=(je == DT - 1))
                for jc in range(CT):
                    nc.vector.tensor_copy(bT[:, jc, :], b_ps[jc][:])
                bias_eff = sm.tile([128, DT], f32, tag="bias_eff", name=f"bias_eff{n}")
                nc.vector.tensor_mul(bias_eff[:], bias_d[:], recip[:])
                st[n]["bT"] = bT
                st[n]["recip"] = recip
                st[n]["bias_eff"] = bias_eff

            def phase_out(n):
                bT, recip, bias_eff = st[n]["bT"], st[n]["recip"], st[n]["bias_eff"]
                xt = st[n]["xt"]
                for jd in range(DT):
                    pieces = [(0, 1, 2), (3, 4, 5, 6)]
                    if n == NPC - 1 and jd == DT - 1:
                        pieces = [(0, 1, 2), (3, 4), (5, 6)]
                    for half, chs in enumerate(pieces):
                        piece = OC * len(chs)
                        off0 = chs[0] * OC
                        o_sb = osb.tile([128, 1792], f32, tag="osb",
                                        name=f"o_sb{n}_{jd}_{half}")
                        for c2, ch in enumerate(chs):
                            o_ps = stg.tile([128, OC], f32, tag="stage",
                                            name=f"o_ps{n}_{jd}_{ch}")
                            for k in range(CT):
                                nc.tensor.matmul(o_ps[:], bT[:, k, jd * 128:(jd + 1) * 128],
                                                 xt[:, k, ch * OC:(ch + 1) * OC],
                                                 start=(k == 0), stop=(k == CT - 1))
                            if ch % 2 == 0:
                                nc.scalar.activation(o_sb[:, c2 * OC:(c2 + 1) * OC], o_ps[:],
                                                     mybir.ActivationFunctionType.Identity,
                                                     bias=bias_eff[:, jd:jd + 1],
                                                     scale=recip[:, jd:jd + 1])
                            else:
                                nc.vector.tensor_scalar(o_sb[:, c2 * OC:(c2 + 1) * OC], o_ps[:],
                                                        recip[:, jd:jd + 1],
                                                        bias_eff[:, jd:jd + 1],
                                                        op0=MUL, op1=ADD)
                            yield
                        nc.sync.dma_start(
                            out=out_ext[n, jd * 128:(jd + 1) * 128,
                                        off0:off0 + piece],
                            in_=o_sb[:, 0:piece])

            # ---- schedule ----
            phase_load_xs(0)
            load_weights()
            phase_load_xt(0)
            for _ in phase_G(0):
                pass
            phase_mid(0)
            phase_load_xs(1)          # ahead of out(0) DMAs on the sync queue
            gO = phase_out(0)
            gG = phase_G(1)
            done_g = False
            for _ in gO:
                for _ in range(4):
                    if next(gG, "END") == "END":
                        done_g = True
                        break
            while not done_g and next(gG, "END") != "END":
                pass
            phase_load_xt(1, halves=True)
            phase_mid(1)
            for _ in phase_out(1):
                pass
    nc.compile()
    return nc


_NC_CACHE = None


def kernel(**inputs: np.ndarray) -> np.ndarray:
    global _NC_CACHE
    from concourse.bass_utils import run_bass_kernel_spmd

    batch = np.asarray(inputs["batch_flat"], dtype=np.float32)
    Wq = np.asarray(inputs["Wq"], dtype=np.float32)
    bq = np.asarray(inputs["bq"], dtype=np.float32)
    Wk = np.asarray(inputs["Wk"], dtype=np.float32)
    bk = np.asarray(inputs["bk"], dtype=np.float32)
    Wv = np.asarray(inputs["Wv"], dtype=np.float32)
    bv = np.asarray(inputs["bv"], dtype=np.float32)

    if _NC_CACHE is None:
        _NC_CACHE = _build_nc()
    nc = _NC_CACHE

    x_r = _f32r_round(batch)
    xs_r = np.ascontiguousarray(x_r.transpose(0, 2, 1))
    m_r = _f32r_round(x_r.astype(np.float64).sum(axis=2).astype(np.float32))
    wqT = _f32r_round(np.concatenate([Wq.T, bq[None, :]], axis=0))
    wkT = _f32r_round(np.concatenate([Wk.T, bk[None, :]], axis=0))
    wv = _f32r_round(Wv)
    bvb = np.ascontiguousarray(np.tile(_f32r_round(bv)[None, :], (128, 1)))
    ident = np.eye(128, dtype=np.float32)

    in_maps = []
    for c in range(NCORES):
        in_maps.append({
            "x": np.ascontiguousarray(x_r[c * NPC:(c + 1) * NPC]),
            "xs": xs_r[c * NPC:(c + 1) * NPC],
            "m": m_r[c * NPC:(c + 1) * NPC],
            "wqT": wqT, "wkT": wkT, "wv": wv, "bvb": bvb, "ident": ident,
            "bkb": np.ascontiguousarray(np.tile(wkT[C:C + 1, :], (128, 1))),
        })
    r = run_bass_kernel_spmd(nc, in_maps, core_ids=list(range(NCORES)))
    out = np.concatenate([r.results[c]["out"] for c in range(NCORES)], axis=0)
    return out.astype(np.float32)


# revision 32
# speedup vs baseline: 1.0290x; 1.0290x over previous
"""Trainium2 Bass kernel for batched channel-attention (nn_Attention_28071906246667).

Reference computation (per batch element n, with xT = batch_flat[n] of shape [C, HW]):
    x   = xT.T                                  # [HW, C]
    Q   = x @ Wq.T + bq ; K, V likewise         # [HW, D]
    S   = Q.T @ K                               # [D, D]
    att = softmax(S, axis=-1)
    out = att @ V.T                             # [D, HW]

Key algebraic restructuring (halves FLOPs, avoids materializing Q/K/V):
    G = x.T x  (Gram over channels), m = column sums of x. Then
      S   = Wq G Wk.T + (Wq m) bk.T + bq (Wk m).T + HW bq bk.T
          = Wq_aug @ U,   U = [G m; m.T HW] @ WkT_aug
      out = att @ V.T = (att_unnorm @ Wv) @ xT + att_unnorm @ bv, normalized at the end.

Sharding: pure data parallel, batch N=16 -> 2 per core across 8 cores.
All matmuls run in float32r (fp32 with 11 explicit mantissa bits, full PE speed).
f32r ISA restrictions honored: moving operand & psum dst innermost counts even,
dst starts at partition 0.
"""

import numpy as np

N, C, HW, D = 16, 512, 3136, 512
NCORES = 8
NPC = N // NCORES          # batch elements per core
CT = C // 128              # 4 c partition tiles
DT = D // 128              # 4 d partition tiles
KT = 25                    # s k-tiles: 24 x 128 + 1 x 64
KT_ROWS = [128] * 24 + [64]
OC = 448                   # out-phase s chunk
NOC = HW // OC             # 7 chunks; DMA'd per 3+4-chunk pieces


def _f32r_round(a: np.ndarray) -> np.ndarray:
    """Round fp32 to float32r (11 explicit mantissa bits, round-to-nearest)."""
    bits = np.ascontiguousarray(a, dtype=np.float32).view(np.uint32)
    bits = (bits + np.uint32(0x800)) & np.uint32(0xFFFFF000)
    return bits.view(np.float32)


def _build_nc():
    import concourse.mybir as mybir
    from concourse import bacc
    from concourse.tile import TileContext

    f32 = mybir.dt.float32
    f32r = mybir.dt.float32r
    MUL = mybir.AluOpType.mult
    ADD = mybir.AluOpType.add

    nc = bacc.Bacc("TRN2", target_bir_lowering=False, debug=False,
                   num_devices=NCORES)

    x_ext = nc.declare_dram_parameter("x", [NPC, C, HW], f32r, isOutput=False)
    xs_ext = nc.declare_dram_parameter("xs", [NPC, HW, C], f32r, isOutput=False)
    m_ext = nc.declare_dram_parameter("m", [NPC, C], f32r, isOutput=False)
    wqT_ext = nc.declare_dram_parameter("wqT", [C + 1, D], f32r, isOutput=False)
    wkT_ext = nc.declare_dram_parameter("wkT", [C + 1, D], f32r, isOutput=False)
    wv_ext = nc.declare_dram_parameter("wv", [D, C], f32r, isOutput=False)
    id_ext = nc.declare_dram_parameter("ident", [128, 128], f32r, isOutput=False)
    bkb_ext = nc.declare_dram_parameter("bkb", [128, D], f32r, isOutput=False)
    bvb_ext = nc.declare_dram_parameter("bvb", [128, D], f32r, isOutput=False)
    out_ext = nc.declare_dram_parameter("out", [NPC, D, HW], f32, isOutput=True)

    with TileContext(nc) as tc:
        with (
            tc.tile_pool(name="wpool", bufs=1) as wp,
            tc.tile_pool(name="xbig", bufs=2) as xb,
            tc.tile_pool(name="work", bufs=1) as wk,
            tc.tile_pool(name="small", bufs=2) as sm,
            tc.tile_pool(name="outsb", bufs=4) as osb,
            tc.tile_pool(name="acc4", bufs=4, space="PSUM") as acc4,
            tc.tile_pool(name="stage", bufs=3, space="PSUM") as stg,
            tc.tile_pool(name="extra", bufs=1, space="PSUM") as xtr,
        ):
            # ---- weights (loaded once) ----
            wq_t = wp.tile([128, CT, D], f32r, tag="wq")
            wq4 = wp.tile([1, D], f32r, tag="wq4")
            wk_t = wp.tile([128, CT, D], f32r, tag="wk")
            wk4 = wp.tile([1, D], f32r, tag="wk4")
            wv_t = wp.tile([128, DT, C], f32r, tag="wv")
            ident = wp.tile([128, 128], f32r, tag="ident")
            bkb = wp.tile([128, D], f32r, tag="bkb")
            bvb = wp.tile([128, D], f32r, tag="bvb")
            hw_t = wp.tile([1, 2], f32r, tag="hw")
            nc.vector.memset(hw_t[:].bitcast(f32), float(HW))

            def load_weights():
                nc.sync.dma_start(out=wk4[:], in_=wkT_ext[C:C + 1, :])
                nc.sync.dma_start(out=wk_t[:],
                                  in_=wkT_ext[0:C, :].rearrange("(ci p) d -> p ci d", p=128))
                nc.sync.dma_start(out=wq_t[:],
                                  in_=wqT_ext[0:C, :].rearrange("(ci p) d -> p ci d", p=128))
                nc.sync.dma_start(out=wq4[:], in_=wqT_ext[C:C + 1, :])
                nc.sync.dma_start(out=bkb[:], in_=bkb_ext[:])
                nc.sync.dma_start(out=ident[:], in_=id_ext[:])
                nc.sync.dma_start(out=wv_t[:],
                                  in_=wv_ext[:].rearrange("(ci p) d -> p ci d", p=128))
                nc.sync.dma_start(out=bvb[:], in_=bvb_ext[:])

            st = {}

            def phase_load_xs(n):
                xs = xb.tile([128, KT, C], f32r, tag="xbig", name=f"xs{n}")
                kt0 = 0
                for ci_, nkt in enumerate([1, 1, 1, 2, 2, 4, 4, 5, 4, 1] if n == 0 else [4, 4, 4, 4, 4, 4, 1]):
                    s0, s1 = kt0 * 128, min((kt0 + nkt) * 128, HW)
                    eng = nc.sync
                    if s1 - s0 == nkt * 128:
                        eng.dma_start(
                            out=xs[:, kt0:kt0 + nkt, :],
                            in_=xs_ext[n, s0:s1, :].rearrange("(k p) c -> p k c", p=128))
                    else:
                        eng.dma_start(out=xs[:s1 - s0, kt0, :],
                                      in_=xs_ext[n, s0:s1, :])
                    kt0 += nkt
                m_r = sm.tile([128, CT], f32r, tag="mr", name=f"mr{n}")
                nc.sync.dma_start(out=m_r[:],
                                  in_=m_ext[n, :].rearrange("(ci c) -> c ci", c=128))
                st[n] = {"xs": xs, "m_r": m_r}

            def phase_load_xt(n, halves=False):
                xt = xb.tile([128, CT, HW], f32r, tag="xbig", name=f"xt{n}")
                if halves:
                    for hoff, hw_ in [(0, 1568), (1568, 1568)]:
                        for ci in range(CT):
                            nc.sync.dma_start(out=xt[:, ci, hoff:hoff + hw_],
                                              in_=x_ext[n, ci * 128:(ci + 1) * 128,
                                                        hoff:hoff + hw_])
                else:
                    for ci in range(CT):
                        nc.sync.dma_start(out=xt[:, ci, :],
                                          in_=x_ext[n, ci * 128:(ci + 1) * 128, :])
                st[n]["xt"] = xt

            G_COLS = {0: (0, 512), 1: (128, 512), 2: (256, 512), 3: (0, 512)}

            def phase_G(n):
                xs = st[n]["xs"]
                g_ps = [acc4.tile([128, 512], f32, tag="acc4", name=f"g_ps{n}_{j}")
                        for j in range(CT)]
                st[n]["g_ps"] = g_ps
                for kt in range(KT):
                    rows = KT_ROWS[kt]
                    for j in range(CT):
                        c0, c1 = G_COLS[j]
                        nc.tensor.matmul(
                            g_ps[j][:, c0:c1],
                            xs[:rows, kt, j * 128:(j + 1) * 128],
                            xs[:rows, kt, c0:c1],
                            start=(kt == 0), stop=(kt == KT - 1))
                        yield

            def phase_mid(n):
                g_ps = st[n]["g_ps"]
                m_r = st[n]["m_r"]
                u4_ps = xtr.tile([1, 512], f32, tag="extra", name=f"u4_ps{n}")
                for k in range(CT):
                    nc.tensor.matmul(u4_ps[:], m_r[:, k:k + 1], wk_t[:, k, :],
                                     start=(k == 0), stop=False)
                nc.tensor.matmul(u4_ps[:], hw_t[0:1, 0:1], wk4[:],
                                 start=False, stop=True)
                g = wk.tile([128, CT, 512], f32r, tag="g", name=f"g{n}")
                for j in range(CT):
                    c0, c1 = G_COLS[j]
                    nc.vector.tensor_copy(g[:, j, c0:c1], g_ps[j][:, c0:c1])
                gfill = stg.tile([128, 512], f32r, tag="stage", name=f"gfill{n}")
                for fi, (bi, bj) in enumerate([(1, 0), (2, 0), (2, 1)]):
                    nc.tensor.transpose(gfill[:, fi * 128:(fi + 1) * 128],
                                        g[:, bj, bi * 128:(bi + 1) * 128], ident[:])
                for fi, (bi, bj) in enumerate([(1, 0), (2, 0), (2, 1)]):
                    nc.vector.tensor_copy(g[:, bi, bj * 128:(bj + 1) * 128],
                                          gfill[:, fi * 128:(fi + 1) * 128])

                # U = G~ @ WkT_aug
                u = wk.tile([128, CT, D], f32r, tag="u", name=f"u{n}")
                u4 = wk.tile([1, D], f32r, tag="u4", name=f"u4{n}")
                u_ps = [acc4.tile([128, 512], f32, tag="acc4", name=f"u_ps{n}_{j}")
                        for j in range(CT)]
                for ki, k in enumerate([0, 3, 1, 2]):
                    for j in range(CT):
                        nc.tensor.matmul(u_ps[j][:], g[:, k, j * 128:(j + 1) * 128],
                                         wk_t[:, k, :], start=(ki == 0), stop=(ki == CT - 1))
                for j in range(CT):
                    nc.vector.scalar_tensor_tensor(
                        u[:, j, :], bkb[:], m_r[:, j:j + 1], u_ps[j][:],
                        op0=MUL, op1=ADD)
                nc.vector.tensor_copy(u4[:], u4_ps[:])

                # S = Wq_aug @ U_aug ; softmax
                s_ps = [acc4.tile([128, 512], f32, tag="acc4", name=f"s_ps{n}_{j}")
                        for j in range(DT)]
                for jd in range(DT):
                    for k in range(CT + 1):
                        lt = wq_t[:, k, :] if k < CT else wq4[:]
                        rhs = u[:, k, :] if k < CT else u4[:]
                        nc.tensor.matmul(s_ps[jd][:], lt[:, jd * 128:(jd + 1) * 128],
                                         rhs, start=(k == 0), stop=(k == CT))

                negmax = sm.tile([128, DT], f32, tag="negmax", name=f"negmax{n}")
                sumexp = sm.tile([128, DT], f32, tag="sumexp", name=f"sumexp{n}")
                recip = sm.tile([128, DT], f32, tag="recip", name=f"recip{n}")
                expS = wk.tile([128, DT, D], f32r, tag="expS", name=f"expS{n}")
                for jd in range(DT):
                    nc.vector.reduce_max(negmax[:, jd:jd + 1], s_ps[jd][:],
                                         axis=mybir.AxisListType.X, negate=True)
                    nc.scalar.activation(expS[:, jd, :], s_ps[jd][:],
                                         mybir.ActivationFunctionType.Exp,
                                         bias=negmax[:, jd:jd + 1], scale=1.0,
                                         accum_out=sumexp[:, jd:jd + 1])
                nc.vector.reciprocal(recip[:], sumexp[:])
                bias_d = sm.tile([128, DT], f32, tag="bias_d", name=f"bias_d{n}")
                bias_dummy = sm.tile([128, 512], f32, tag="bias_dummy",
                                     name=f"bias_dummy{n}")
                for jd in range(DT):
                    nc.vector.scalar_tensor_tensor(
                        bias_dummy[:], expS[:, jd, :], 1.0, bvb[:],
                        op0=MUL, op1=MUL, accum_out=bias_d[:, jd:jd + 1])

                # attT via transposes
                attT = wk.tile([128, DT, D], f32r, tag="attT", name=f"attT{n}")
                for je in range(DT):
                    at = stg.tile([128, 512], f32r, tag="stage", name=f"at{n}_{je}")
                    for jd in range(DT):
                        nc.tensor.transpose(at[:, jd * 128:(jd + 1) * 128],
                                            expS[:, jd, je * 128:(je + 1) * 128],
                                            ident[:])
                    nc.vector.tensor_copy(attT[:, je, :], at[:])

                # B^T = Wv^T @ attT
                bT = wk.tile([128, CT, D], f32r, tag="bT", name=f"bT{n}")
                b_ps = [acc4.tile([128, 512], f32, tag="acc4", name=f"b_ps{n}_{j}")
                        for j in range(CT)]
                for je in range(DT):
                    for jc in range(CT):
                        nc.tensor.matmul(b_ps[jc][:], wv_t[:, je, jc * 128:(jc + 1) * 128],
                                         attT[:, je, :], start=(je == 0), stop=(je == DT - 1))
                for jc in range(CT):
                    nc.vector.tensor_copy(bT[:, jc, :], b_ps[jc][:])
                bias_eff = sm.tile([128, DT], f32, tag="bias_eff", name=f"bias_eff{n}")
                nc.vector.tensor_mul(bias_eff[:], bias_d[:], recip[:])
                st[n]["bT"] = bT
                st[n]["recip"] = recip
                st[n]["bias_eff"] = bias_eff

            def phase_out(n):
                bT, recip, bias_eff = st[n]["bT"], st[n]["recip"], st[n]["bias_eff"]
                xt = st[n]["xt"]
                for jd in range(DT):
                    pieces = [(0, 1, 2), (3, 4, 5, 6)]
                    if n == NPC - 1 and jd == DT - 1:
                        pieces = [(0, 1, 2), (3, 4), (5, 6)]
                    for half, chs in enumerate(pieces):
                        piece = OC * len(chs)
                        off0 = chs[0] * OC
                        o_sb = osb.tile([128, 1792], f32, tag="osb",
                                        name=f"o_sb{n}_{jd}_{half}")
                        for c2, ch in enumerate(chs):
                            o_ps = stg.tile([128, OC], f32, tag="stage",
                                            name=f"o_ps{n}_{jd}_{ch}")
                            for k in range(CT):
                                nc.tensor.matmul(o_ps[:], bT[:, k, jd * 128:(jd + 1) * 128],
                                                 xt[:, k, ch * OC:(ch + 1) * OC],
                                                 start=(k == 0), stop=(k == CT - 1))
                            if ch % 2 == 0:
                                nc.scalar.activation(o_sb[:, c2 * OC:(c2 + 1) * OC], o_ps[:],
                                                     mybir.ActivationFunctionType.Identity,
                                                     bias=bias_eff[:, jd:jd + 1],
                                                     scale=recip[:, jd:jd + 1])
                            else:
                                nc.vector.tensor_scalar(o_sb[:, c2 * OC:(c2 + 1) * OC], o_ps[:],
                                                        recip[:, jd:jd + 1],
                                                        bias_eff[:, jd:jd + 1],
                                                        op0=MUL, op1=ADD)
                            yield
                        nc.sync.dma_start(
                            out=out_ext[n, jd * 128:(jd + 1) * 128,
                                        off0:off0 + piece],
                            in_=o_sb[:, 0:piece])

            # ---- schedule ----
            phase_load_xs(0)
            load_weights()
            phase_load_xt(0)
            for _ in phase_G(0):
                pass
            phase_mid(0)
            phase_load_xs(1)          # ahead of out(0) DMAs on the sync queue
            gO = phase_out(0)
            gG = phase_G(1)
            done_g = False
            for _ in gO:
                for _ in range(4):
                    if next(gG, "END") == "END":
                        done_g = True
                        break
            while not done_g and next(gG, "END") != "END":
                pass
            phase_load_xt(1, halves=True)
            phase_mid(1)
            for _ in phase_out(1):
                pass
    nc.compile()
    return nc


_NC_CACHE = None


def kernel(**inputs: np.ndarray) -> np.ndarray:
    global _NC_CACHE
    from concourse.bass_utils import run_bass_kernel_spmd

    batch = np.asarray(inputs["batch_flat"], dtype=np.float32)
    Wq = np.asarray(inputs["Wq"], dtype=np.float32)
    bq = np.asarray(inputs["bq"], dtype=np.float32)
    Wk = np.asarray(inputs["Wk"], dtype=np.float32)
    bk = np.asarray(inputs["bk"], dtype=np.float32)
    Wv = np.asarray(inputs["Wv"], dtype=np.float32)
    bv = np.asarray(inputs["bv"], dtype=np.float32)

    if _NC_CACHE is None:
        _NC_CACHE = _build_nc()
    nc = _NC_CACHE

    x_r = _f32r_round(batch)
    xs_r = np.ascontiguousarray(x_r.transpose(0, 2, 1))
    m_r = _f32r_round(x_r.astype(np.float64).sum(axis=2).astype(np.float32))
    wqT = _f32r_round(np.concatenate([Wq.T, bq[None, :]], axis=0))
    wkT = _f32r_round(np.concatenate([Wk.T, bk[None, :]], axis=0))
    wv = _f32r_round(Wv)
    bvb = np.ascontiguousarray(np.tile(_f32r_round(bv)[None, :], (128, 1)))
    ident = np.eye(128, dtype=np.float32)

    in_maps = []
    for c in range(NCORES):
        in_maps.append({
            "x": np.ascontiguousarray(x_r[c * NPC:(c + 1) * NPC]),
            "xs": xs_r[c * NPC:(c + 1) * NPC],
            "m": m_r[c * NPC:(c + 1) * NPC],
            "wqT": wqT, "wkT": wkT, "wv": wv, "bvb": bvb, "ident": ident,
            "bkb": np.ascontiguousarray(np.tile(wkT[C:C + 1, :], (128, 1))),
        })
    r = run_bass_kernel_spmd(nc, in_maps, core_ids=list(range(NCORES)))
    out = np.concatenate([r.results[c]["out"] for c in range(NCORES)], axis=0)
    return out.astype(np.float32)
